# revision 2
# baseline (speedup 1.0000x reference)
"""ConvNeXt block kernel for Trainium2 (8 NeuronCores, data-parallel over batch).

v2: fp8 DoubleRow everywhere.
  y = x + gamma * ( GELU( LN(dwconv7x7(x) + dw_b) @ w1 + b1 ) @ w2 + b2 )

Per-core: 2 images, channels on partitions (3 blocks of 128), pixels free.

Depthwise conv 7x7: fp8 DoubleRow diagonal matmuls pairing 2 taps per
instruction (pair = two shifted views of the same padded image, dim1 stride 1).
Rows are split PE / GPSIMD / DVE per REGIONS; the GPSIMD+DVE parts run
scalar_tensor_tensor chains on a bf16 copy of the padded image.

MLP: fp8 DoubleRow, 2 matmuls per pwconv1 block (cb0+cb1, cb2+mean-row) and
6 per pwconv2 block (12 hidden blocks paired). LN folded: rstd multiplied into
x (bf16 -> fp8), mean correction rides the second pwconv1 matmul's pair slot
on partition 0, b1 via GELU bias, w2 scale + gamma folded into the output
scalar_tensor_tensor, gamma*b2 pre-added to the residual input on the host.

Weight scales (fp8 ranges): conv weights x64 (undone in the PSUM->SBUF copy),
w1 x32 (undone in the GELU scale), w2 x32 (undone in the output stt), mean
row: mu*rstd x16 and s1 x2.
"""

import sys

sys.path.insert(0, "/opt/trn_rl_repo")

import numpy as np
import ml_dtypes

import bass_rust
import concourse.bass as bass
import concourse.mybir as mybir
import concourse.tile as tile
from concourse.bass_utils import run_bass_kernel_spmd

F32 = mybir.dt.float32
BF16 = mybir.dt.bfloat16
FP8 = mybir.dt.float8e4
I32 = mybir.dt.int32
AF = mybir.ActivationFunctionType
ALU = mybir.AluOpType
DR = mybir.MatmulPerfMode.DoubleRow

N_CORES = 8
IMGS_PER_CORE = 2
C = 384
CB = 3
H = W = 56
PIX = H * W              # 3136
WPAD = 62
GLO = 3                  # head guard elements
GHI = 13                 # tail guard; also pads XLEN to a 16 multiple (DR pair
                         # stride must be 16B-aligned: checkMatmultPerfMode)
XLEN = GLO + H * WPAD + GHI
assert XLEN % 16 == 0
CHUNK = 448              # pixels per MLP chunk (8 rows)
NCHUNK = 7
FD = 1536
NFC = 12
EPS = 1e-6

SW = 64.0                # conv weight fp8 scale
S1 = 32.0                # w1 fp8 scale
S2 = 32.0                # w2 fp8 scale
SMU = 16.0               # mu*rstd scale (s1 row scaled by S1/SMU)
GAMMA = 1e-6             # layer scale (uniform, folded into output stt)

# per image: rows [0, r_pe) on PE (fp8 DoubleRow pairs), [r_pe, r_ap) on
# ACT products + Pool adds, [r_ap, 56) on DVE stt. All multiples of 8.
import os
REGIONS = {0: (48, 48), 1: (40, 40)}
DBG_NO_CONV_PE = bool(int(os.environ.get("DBG_NO_CONV_PE", "0")))
DBG_NO_MLP = bool(int(os.environ.get("DBG_NO_MLP", "0")))
DBG_CONV_FLATCOPY = bool(int(os.environ.get("DBG_CONV_FLATCOPY", "0")))
DBG_CONV_D0ONLY = bool(int(os.environ.get("DBG_CONV_D0ONLY", "0")))

MAGIC = 0x5F3759DF

_WAITSPLIT_N = [0]


def _split_waits(nc, max_waits=1):
    """This walrus build rejects instructions with more than one sync-wait
    command; hoist excess waits onto dedicated NoOps on the same engine."""
    for fn in nc.m.functions:
        for bb in fn.blocks:
            insts = bb.instructions
            idx = 0
            while idx < len(insts):
                ins = insts[idx]
                si = ins.sync_info
                if si is not None and len(si.on_wait) > max_waits:
                    waits = list(si.on_wait)
                    extra, keep = waits[:-max_waits], waits[-max_waits:]
                    nops = []
                    for w in extra:
                        _WAITSPLIT_N[0] += 1
                        nops.append(
                            mybir.InstNoOp(
                                name=f"I-wsplit-{_WAITSPLIT_N[0]}",
                                engine=ins.engine,
                                ins=[],
                                outs=[],
                                sync_info=bass_rust.SyncInfo(
                                    on_wait=[w], on_update=[]
                                ),
                            )
                        )
                    ins.sync_info = bass_rust.SyncInfo(
                        on_wait=keep, on_update=list(si.on_update)
                    )
                    insts[idx:idx] = nops
                    idx += len(nops)
                idx += 1


def _paired_view(tile_ap, off, pair_stride, n):
    """[128, 2, n] read view of a [128, F] tile at element offset off; the
    pair (DoubleRow k-tile) dim has stride pair_stride elements."""
    c = tile_ap[:, off : off + 1].copy()
    pstride = c.ap.to_list()[0][0]
    c.ap = bass_rust.VecI64Pair([[pstride, 128], [pair_stride, 2], [1, n]])
    return c


# tap pairs per kernel row: (e0 of slot0); slot1 = e0+1. j=3 pairs (e=3, dummy)
PAIR_E0 = (-3, -1, 1, 3)


def _build_nc(n_imgs=IMGS_PER_CORE, split_waits=True):
    nc = bass.Bass(trn_type="TRN2", target_bir_lowering=False, debug=False)

    xp8 = nc.dram_tensor("xp8", [IMGS_PER_CORE, CB, 128, 2 * XLEN], FP8, kind="ExternalInput")
    xp16 = nc.dram_tensor("xp16", [IMGS_PER_CORE, CB, 128, XLEN], BF16, kind="ExternalInput")
    wdiag = nc.dram_tensor("wdiag", [128, CB * 7 * 4 * 2 * 128], FP8, kind="ExternalInput")
    wt = nc.dram_tensor("wt", [128, CB * 49], F32, kind="ExternalInput")
    dwb = nc.dram_tensor("dwb", [128, CB], F32, kind="ExternalInput")
    w1q = nc.dram_tensor("w1q", [128, 2 * NFC * 128], FP8, kind="ExternalInput")
    w1b = nc.dram_tensor("w1b", [128, 2 * NFC * 128], FP8, kind="ExternalInput")
    w2q = nc.dram_tensor("w2q", [128, 2 * 6 * C], FP8, kind="ExternalInput")
    b1p = nc.dram_tensor("b1p", [128, NFC], F32, kind="ExternalInput")
    gok = nc.dram_tensor("gok", [128, CB], F32, kind="ExternalInput")
    xs = nc.dram_tensor("xs", [IMGS_PER_CORE, C, PIX], F32, kind="ExternalInput")
    ys = nc.dram_tensor("ys", [IMGS_PER_CORE, C, PIX], F32, kind="ExternalOutput")
    vscratch = nc.dram_tensor("vscratch", [IMGS_PER_CORE, PIX], F32, kind="Internal")
    rscratch = nc.dram_tensor("rscratch", [IMGS_PER_CORE, PIX], F32, kind="Internal")

    with tile.TileContext(nc) as tc:
        with (
            tc.tile_pool(name="const", bufs=1) as constp,
            tc.tile_pool(name="xpad8", bufs=6) as xp8p,
            tc.tile_pool(name="xpad16", bufs=6) as xp16p,
            tc.tile_pool(name="acc", bufs=3) as accp,
            tc.tile_pool(name="prod", bufs=3) as prodp,
            tc.tile_pool(name="xt", bufs=2) as xtp,
            tc.tile_pool(name="h", bufs=2) as hp,
            tc.tile_pool(name="small", bufs=2) as smallp,
            tc.tile_pool(name="stat", bufs=2) as statp,
            tc.tile_pool(name="outp", bufs=2) as outp,
            tc.tile_pool(name="ps1", bufs=2, space="PSUM") as ps1p,
            tc.tile_pool(name="ps2", bufs=2, space="PSUM") as ps2p,
            tc.tile_pool(name="psstat", bufs=2, space="PSUM") as psstatp,
            tc.tile_pool(name="pstap", bufs=2, space="PSUM") as pstapp,
        ):
            # ---- static weights ----
            wdiag_sb = constp.tile([128, CB, 7, 4, 2, 128], FP8)
            nc.sync.dma_start(wdiag_sb[:], wdiag.ap().rearrange(
                "p (cb d j i f) -> p cb d j i f", cb=CB, d=7, j=4, i=2))
            wt_sb = constp.tile([128, CB, 49], F32)
            nc.sync.dma_start(wt_sb[:], wt.ap().rearrange("p (cb t) -> p cb t", cb=CB))
            dwb_sb = constp.tile([128, CB], F32)
            nc.sync.dma_start(dwb_sb[:], dwb.ap())
            w1q_sb = constp.tile([128, 2, NFC, 128], FP8)
            nc.sync.dma_start(w1q_sb[:], w1q.ap().rearrange(
                "p (i fc f) -> p i fc f", i=2, fc=NFC))
            w1b_sb = constp.tile([128, 2, NFC, 128], FP8)
            nc.sync.dma_start(w1b_sb[:], w1b.ap().rearrange(
                "p (i fc f) -> p i fc f", i=2, fc=NFC))
            w2q_sb = constp.tile([128, 2, 6, C], FP8)
            nc.sync.dma_start(w2q_sb[:], w2q.ap().rearrange(
                "p (i jj c) -> p i jj c", i=2, jj=6))
            b1_sb = constp.tile([128, NFC], F32)
            nc.sync.dma_start(b1_sb[:], b1p.ap())
            gok_sb = constp.tile([128, CB], F32)
            nc.sync.dma_start(gok_sb[:], gok.ap())
            ones_bf = constp.tile([128, 1], BF16)
            nc.vector.memset(ones_bf[:], 1.0)

            taps = [(d, e) for d in range(-3, 4) for e in range(-3, 4)
                    if not (d == 0 and e == 0)]

            accs = {}  # (img, cb) -> [(r0, r1, acc_tile), ...]

            def acc_chunk(img, cb, ch):
                r = ch * 8
                for r0, r1, t in accs[(img, cb)]:
                    if r0 <= r < r1:
                        return t[:, (r - r0) * W : (r + 8 - r0) * W]
                raise AssertionError

            for img in range(n_imgs):
                r_pe, r_ap = REGIONS[img]
                for cb in range(CB):
                    regs = []
                    # ---------- PE region: rows [0, r_pe), fp8 DR tap pairs ----------
                    # dual-plane image: plane 1 = plane 0 shifted left one elem,
                    # so a tap pair (e, e+1) is one [128, 2, n] slot AP.
                    xpad8 = xp8p.tile([128, 2, XLEN], FP8, tag="xpad8")
                    nc.sync.dma_start(
                        xpad8[:], xp8.ap()[img, cb].rearrange("p (i n) -> p i n", i=2))
                    acc_pe = accp.tile([128, r_pe * W], BF16, tag=f"accpe{r_pe}")
                    if DBG_NO_CONV_PE:
                        nc.vector.memset(acc_pe[:], 0.5)
                    for hc0 in range(0, r_pe, 8) if not DBG_NO_CONV_PE else []:
                        pst = pstapp.tile([128, 8 * WPAD], F32, tag="pstap")
                        work = []
                        dlist = (3,) if DBG_CONV_D0ONLY else (3, 0, 1, 2, 4, 5, 6)
                        for d in dlist:  # d_off = d-3; full row first
                            d_off = d - 3
                            hs = max(hc0, -d_off)
                            he = min(hc0 + 8, H - d_off)
                            if he <= hs:
                                continue
                            for j in range(4):
                                work.append((d, d_off, j, hs, he))
                        for k, (d, d_off, j, hs, he) in enumerate(work):
                            off = GLO + (hs + d_off) * WPAD + PAIR_E0[j]
                            rhs = xpad8[:, :, off : off + (he - hs) * WPAD]
                            nc.tensor.matmul(
                                pst[:, (hs - hc0) * WPAD : (he - hc0) * WPAD],
                                wdiag_sb[:, cb, d, j, :, :],
                                rhs,
                                start=(k == 0),
                                stop=(k == len(work) - 1),
                                perf_mode=DR,
                            )
                        pst3 = pst.rearrange("p (h w) -> p h w", w=WPAD)
                        if DBG_CONV_FLATCOPY:
                            nc.scalar.activation(
                                acc_pe[:, hc0 * W : (hc0 + 8) * W],
                                pst[:, 0 : 8 * W],
                                AF.Identity,
                                bias=dwb_sb[:, cb : cb + 1],
                                scale=1.0 / SW,
                            )
                        else:
                            nc.scalar.activation(
                                acc_pe[:, hc0 * W : (hc0 + 8) * W],
                                pst3[:, :, GLO : GLO + W],
                                AF.Identity,
                                bias=dwb_sb[:, cb : cb + 1],
                                scale=1.0 / SW,
                            )
                    regs.append((0, r_pe, acc_pe))

                    # ---------- ACT+Pool / DVE regions on bf16 ----------
                    def conv_region(r0, r1, mode, tag):
                        """mode 'dve': scalar_tensor_tensor chain on DVE.
                        mode 'actpool': ACT products, Pool tensor_tensor adds."""
                        if r1 <= r0:
                            return
                        base_in = r0 - 3  # first input row held in the tile
                        rows_in = min(r1 + 3, H) - base_in
                        xpad16 = xp16p.tile([128, rows_in * WPAD], BF16, tag=f"x16{tag}")
                        nc.sync.dma_start(
                            xpad16[:],
                            xp16.ap()[img, cb, :, GLO + base_in * WPAD : GLO + (base_in + rows_in) * WPAD],
                        )
                        x3 = xpad16.rearrange("p (h w) -> p h w", w=WPAD)
                        acc = accp.tile([128, (r1 - r0) * W], BF16, tag=f"acc{tag}{r1 - r0}")
                        a3 = acc.rearrange("p (h w) -> p h w", w=W)
                        nc.scalar.activation(
                            a3[:],
                            x3[:, r0 - base_in : r1 - base_in, 3 : 3 + W],
                            AF.Identity,
                            bias=dwb_sb[:, cb : cb + 1],
                            scale=wt_sb[:, cb, 24:25],
                        )
                        for d, e in taps:
                            hs = max(r0, -d)
                            he = min(r1, H - d)
                            if he <= hs:
                                continue
                            t = (d + 3) * 7 + (e + 3)
                            src = x3[:, hs + d - base_in : he + d - base_in, 3 + e : 3 + e + W]
                            dst = a3[:, hs - r0 : he - r0, :]
                            if mode == "dve":
                                nc.vector.scalar_tensor_tensor(
                                    out=dst, in0=src,
                                    scalar=wt_sb[:, cb, t : t + 1],
                                    in1=dst, op0=ALU.mult, op1=ALU.add,
                                )
                            else:
                                prod = prodp.tile(
                                    [128, (r1 - r0) * W], BF16, tag=f"prod{tag}")
                                p3 = prod.rearrange("p (h w) -> p h w", w=W)
                                nc.scalar.activation(
                                    p3[:, : he - hs, :], src, AF.Identity,
                                    scale=wt_sb[:, cb, t : t + 1],
                                )
                                nc.gpsimd.tensor_tensor(
                                    dst, dst, p3[:, : he - hs, :], ALU.add)
                        regs.append((r0, r1, acc))

                    conv_region(r_pe, r_ap, "actpool", "g")
                    conv_region(r_ap, H, "dve", "d")
                    accs[(img, cb)] = regs

                # ------------- chunk-pipelined LN stats + rstd + MLP -------------
                for ch in range(NCHUNK):
                    _hp = tc.high_priority(offset=500000)
                    _hp.__enter__()
                    sl = slice(ch * CHUNK, (ch + 1) * CHUNK)
                    pmu = psstatp.tile([1, CHUNK], F32, tag="ps_stat")
                    for cb in range(CB):
                        nc.tensor.matmul(
                            pmu[:], ones_bf[:, 0:1], acc_chunk(img, cb, ch),
                            start=(cb == 0), stop=(cb == CB - 1),
                        )
                    mu32 = smallp.tile([1, CHUNK], F32, tag="mu32")
                    nc.vector.tensor_scalar_mul(mu32[:], pmu[:], 1.0 / C)
                    pmsq = psstatp.tile([1, CHUNK], F32, tag="ps_stat")
                    for cb in range(CB):
                        ysq = statp.tile([128, CHUNK], BF16, tag="ysq")
                        nc.scalar.activation(ysq[:], acc_chunk(img, cb, ch), AF.Square)
                        nc.tensor.matmul(
                            pmsq[:], ones_bf[:, 0:1], ysq[:],
                            start=(cb == 0), stop=(cb == CB - 1),
                        )
                    tq = smallp.tile([1, CHUNK], F32, tag="tq")
                    nc.vector.tensor_mul(tq[:], mu32[:], mu32[:])
                    vchunk = smallp.tile([1, CHUNK], F32, tag="vchunk")
                    nc.vector.scalar_tensor_tensor(
                        out=vchunk[:], in0=pmsq[:], scalar=1.0 / C, in1=tq[:],
                        op0=ALU.mult, op1=ALU.subtract,
                    )
                    nc.sync.dma_start(vscratch.ap()[img : img + 1, sl], vchunk[0:1, :])

                    # per-chunk Newton rsqrt in [56,8] transposed layout
                    vpf = smallp.tile([56, 8], F32, tag="vpf")
                    nc.sync.dma_start(vpf[:], vscratch.ap()[img, sl].rearrange("(p f) -> p f", p=56))
                    v_eps = smallp.tile([56, 8], F32, tag="veps")
                    nc.vector.tensor_scalar_add(v_eps[:], vpf[:], EPS)
                    yr = smallp.tile([56, 8], F32, tag="yr")
                    ti = smallp.tile([56, 8], I32, tag="ti")
                    nc.vector.tensor_scalar(
                        ti[:], v_eps[:].bitcast(I32), 1, None, ALU.logical_shift_right
                    )
                    nc.vector.tensor_scalar(ti[:], ti[:], -1, None, ALU.bitwise_xor)
                    nc.vector.tensor_scalar(yr[:].bitcast(I32), ti[:], MAGIC + 1, None, ALU.add)
                    rr = smallp.tile([56, 8], F32, tag="rr")
                    for _ in range(3):
                        nc.vector.tensor_mul(rr[:], yr[:], yr[:])
                        nc.vector.tensor_mul(rr[:], rr[:], v_eps[:])
                        nc.vector.tensor_scalar(rr[:], rr[:], -0.5, 1.5, ALU.mult, ALU.add)
                        nc.vector.tensor_mul(yr[:], yr[:], rr[:])
                    nc.sync.dma_start(
                        rscratch.ap()[img, sl].rearrange("(p f) -> p f", p=56), yr[:]
                    )
                    rb = xtp.tile([128, CHUNK], BF16, tag="rb")
                    nc.gpsimd.dma_start(rb[:], rscratch.ap()[img, sl].partition_broadcast(128))

                    # normalized fp8 inputs: xq01 = (cb0, cb1), xq2m = (cb2, mean row)
                    xq01 = xtp.tile([128, 2, CHUNK], FP8, tag="xq01")
                    nc.vector.tensor_mul(xq01[:, 0, :], acc_chunk(img, 0, ch), rb[:])
                    nc.vector.tensor_mul(xq01[:, 1, :], acc_chunk(img, 1, ch), rb[:])
                    xq2m = xtp.tile([128, 2, CHUNK], FP8, tag="xq2m")
                    nc.vector.tensor_mul(xq2m[:, 0, :], acc_chunk(img, 2, ch), rb[:])
                    nc.gpsimd.memset(xq2m[:, 1, :], 0.0)
                    nc.vector.scalar_tensor_tensor(
                        out=xq2m[0:1, 1, :], in0=mu32[:], scalar=SMU,
                        in1=rb[0:1, :], op0=ALU.mult, op1=ALU.mult,
                    )

                    hblk = hp.tile([128, NFC, CHUNK], FP8, tag="h")
                    if DBG_NO_MLP:
                        nc.vector.memset(hblk[:], 0.25)
                    for fc in range(NFC) if not DBG_NO_MLP else []:
                        p1 = ps1p.tile([128, CHUNK], F32, tag="p1")
                        nc.tensor.matmul(
                            p1[:], w1q_sb[:, :, fc, :], xq01[:],
                            start=True, stop=False, perf_mode=DR,
                        )
                        nc.tensor.matmul(
                            p1[:], w1b_sb[:, :, fc, :], xq2m[:],
                            start=False, stop=True, perf_mode=DR,
                        )
                        nc.scalar.activation(
                            hblk[:, fc, :], p1[:], AF.Gelu,
                            bias=b1_sb[:, fc : fc + 1], scale=1.0 / S1,
                        )
                    for cb in range(CB):
                        cs = slice(cb * 128, (cb + 1) * 128)
                        p2 = ps2p.tile([128, CHUNK], F32, tag="p2")
                        for jj in range(6):
                            nc.tensor.matmul(
                                p2[:], w2q_sb[:, :, jj, cs], hblk[:, 2 * jj : 2 * jj + 2, :],
                                start=(jj == 0), stop=(jj == 5), perf_mode=DR,
                            )
                        xres = outp.tile([128, CHUNK], F32, tag="xres")
                        nc.sync.dma_start(xres[:], xs.ap()[img, cs, sl])
                        osb = outp.tile([128, CHUNK], F32, tag="osb")
                        nc.vector.scalar_tensor_tensor(
                            out=osb[:], in0=p2[:], scalar=gok_sb[:, cb : cb + 1],
                            in1=xres[:], op0=ALU.mult, op1=ALU.add,
                        )
                        nc.sync.dma_start(ys.ap()[img, cs, sl], osb[:])
                    _hp.__exit__(None, None, None)

    if split_waits:
        _split_waits(nc)
    return nc


_NC_CACHE = None


def _host_fold(inputs):
    dw_w = np.asarray(inputs["dw_w"], dtype=np.float32)
    dw_b = np.asarray(inputs["dw_b"], dtype=np.float32)
    ln_w = np.asarray(inputs["ln_w"], dtype=np.float32)
    ln_b = np.asarray(inputs["ln_b"], dtype=np.float32)
    w1 = np.asarray(inputs["w1"], dtype=np.float32)
    b1 = np.asarray(inputs["b1"], dtype=np.float32)
    w2 = np.asarray(inputs["w2"], dtype=np.float32)
    b2 = np.asarray(inputs["b2"], dtype=np.float32)
    gamma = np.asarray(inputs["gamma"], dtype=np.float32)

    def q8(a):
        return np.clip(a, -240.0, 240.0).astype(ml_dtypes.float8_e4m3)

    # conv weights: [7,7,1,C] -> [C, 7, 7]
    wtap = dw_w[:, :, 0, :].transpose(2, 0, 1)          # [C, d, e]
    wdiag = np.zeros((128, CB, 7, 4, 2, 128), np.float32)
    idx = np.arange(128)
    wq = q8(SW * wtap).astype(np.float32)               # quantized, scaled
    for cb in range(CB):
        for d in range(7):
            for j in range(4):
                for i in range(2):
                    e = PAIR_E0[j] + i
                    if e > 3:
                        continue  # dummy slot of the (e=3, -) pair stays 0
                    wdiag[idx, cb, d, j, i, idx] = wq[cb * 128 + idx, d, e + 3]
    wdiag8 = wdiag.astype(ml_dtypes.float8_e4m3).reshape(128, -1)

    wt = wtap.reshape(C, 49).reshape(CB, 128, 49).transpose(1, 0, 2).reshape(128, -1)
    dwbm = dw_b.reshape(CB, 128).T.copy()

    w1p = (ln_w[:, None] * w1).astype(np.float32)       # LN scale into w1
    b1f = (b1 + ln_b @ w1).astype(np.float32)
    w1q8 = q8(S1 * w1p)                                  # [C, FD] fp8
    w1qf = w1q8.astype(np.float32)
    s1n = -(w1qf.sum(axis=0) / S1)                       # [FD], from quantized w1
    # stationary layouts: [p, i, fc, f]
    w1q_arr = np.zeros((128, 2, NFC, 128), ml_dtypes.float8_e4m3)
    w1b_arr = np.zeros((128, 2, NFC, 128), ml_dtypes.float8_e4m3)
    for fc in range(NFC):
        fs = slice(fc * 128, (fc + 1) * 128)
        w1q_arr[:, 0, fc, :] = w1q8[0:128, fs]
        w1q_arr[:, 1, fc, :] = w1q8[128:256, fs]
        w1b_arr[:, 0, fc, :] = w1q8[256:384, fs]
        w1b_arr[0, 1, fc, :] = q8((S1 / SMU) * s1n[fs])
    w2q8 = q8(S2 * w2)                                   # [FD, C]
    w2q_arr = np.zeros((128, 2, 6, C), ml_dtypes.float8_e4m3)
    for jj in range(6):
        w2q_arr[:, 0, jj, :] = w2q8[(2 * jj) * 128 : (2 * jj + 1) * 128, :]
        w2q_arr[:, 1, jj, :] = w2q8[(2 * jj + 1) * 128 : (2 * jj + 2) * 128, :]

    b1m = b1f.reshape(NFC, 128).T.copy()

    return {
        "wdiag": wdiag8,
        "wt": wt.astype(np.float32),
        "dwb": dwbm.astype(np.float32),
        "w1q": w1q_arr.reshape(128, -1),
        "w1b": w1b_arr.reshape(128, -1),
        "w2q": w2q_arr.reshape(128, -1),
        "b1p": b1m.astype(np.float32),
        "gok": (gamma.reshape(CB, 128).T / S2).astype(np.float32),
        "gb2": (gamma * b2).astype(np.float32),
    }


def make_in_maps(inputs):
    x = np.asarray(inputs["x"], dtype=np.float32)
    common = _host_fold(inputs)
    gb2 = common.pop("gb2")

    # padded image in both precisions: [16, C, H, WPAD] with zero W pads
    xpad = np.zeros((16, C, H, WPAD), np.float32)
    xpad[:, :, :, 3 : 3 + W] = x
    xflat = xpad.reshape(16, C, H * WPAD)
    xg = np.zeros((16, C, XLEN), np.float32)
    xg[:, :, GLO : GLO + H * WPAD] = xflat
    xg8 = np.clip(xg, -240.0, 240.0).astype(ml_dtypes.float8_e4m3)
    # dual-plane fp8: plane1 shifted left by one element
    xg8d = np.zeros((16, C, 2, XLEN), ml_dtypes.float8_e4m3)
    xg8d[:, :, 0, :] = xg8
    xg8d[:, :, 1, :-1] = xg8[:, :, 1:]
    xg16 = xg.astype(ml_dtypes.bfloat16)

    xres = (x + gb2[None, :, None, None]).reshape(16, C, PIX)

    in_maps = []
    for k in range(N_CORES):
        m = dict(common)
        sl = slice(k * IMGS_PER_CORE, (k + 1) * IMGS_PER_CORE)
        m["xp8"] = np.ascontiguousarray(
            xg8d[sl].reshape(IMGS_PER_CORE, CB, 128, 2 * XLEN))
        m["xp16"] = np.ascontiguousarray(
            xg16[sl].reshape(IMGS_PER_CORE, CB, 128, XLEN))
        m["xs"] = np.ascontiguousarray(xres[sl])
        in_maps.append(m)
    return in_maps


def kernel(**inputs):
    global _NC_CACHE
    in_maps = make_in_maps(inputs)
    if _NC_CACHE is None:
        _NC_CACHE = _build_nc()
    res = run_bass_kernel_spmd(_NC_CACHE, in_maps, core_ids=list(range(N_CORES)))
    out = np.concatenate([res.results[k]["ys"] for k in range(N_CORES)], axis=0)
    return out.reshape(16, C, H, W).astype(np.float32)


if __name__ == "__main__":
    rng = np.random.default_rng(0)
    ins = {
        "x": rng.standard_normal((16, C, H, W), dtype=np.float32),
        "dw_w": 0.02 * rng.standard_normal((7, 7, 1, C), dtype=np.float32),
        "dw_b": 0.02 * rng.standard_normal((C,), dtype=np.float32),
        "ln_w": np.ones(C, np.float32),
        "ln_b": np.zeros(C, np.float32),
        "w1": (C**-0.5) * rng.standard_normal((C, FD), dtype=np.float32),
        "b1": 0.02 * rng.standard_normal((FD,), dtype=np.float32),
        "w2": ((4 * C) ** -0.5) * rng.standard_normal((FD, C), dtype=np.float32),
        "b2": 0.02 * rng.standard_normal((C,), dtype=np.float32),
        "gamma": np.full((C,), 1e-6, np.float32),
    }
    out = kernel(**ins)
    print("out", out.shape, out.dtype, np.abs(out).mean())
